# revision 26
# baseline (speedup 1.0000x reference)
"""Trainium2 Bass kernel for nn_Network_80049600463282.

LSTM language model: B=16, T=512, V=4096 (one-hot input), H=512 LSTM,
then MLP 512 -> 200 (relu) -> 4096, with fixed batch-norm scaling.

Strategy (8 NeuronCores, SPMD, zero collectives):
  - Data-parallel over batch: 2 examples per core.
  - One-hot @ W_x == embedding row gather -> precomputed host-side into the
    exact transposed SBUF layout the recurrence consumes (lstm bias + forget
    bias folded in).
  - Recurrence in transposed layout (gate dims on partitions): per step,
    64 bf16 matmuls [128x128 stationary W_h tile] x [128x2 moving h^T],
    PSUM-accumulated per 128-dim output chunk; sigmoid/tanh on ScalarE,
    cell update on VectorE, chunked x4 so gate latency hides under the
    next chunk's weight streaming.
  - hs^T accumulates in SBUF in matmul-ready layout; MLP runs at the end
    with BN scales folded into W1/W2 and b2 folded in via K=201 augmentation.
"""

import os
import numpy as np
import ml_dtypes

V = 4096
B = 16
T = int(os.environ.get("LSTM_KERNEL_T", "512"))
H = 512
DH = 200
N_CORES = 8
BL = 2  # examples per core
BN_S = 1.0 / np.sqrt(1.0 + 0.001)
# W_h is stored fp8e4m3 (FWL streams fp8 weights 4 cols/cycle vs bf16's 2).
# Weights and the E term are pre-scaled by WH_SCALE so fp8 stays in its
# normal range; the sigmoid applies 1/WH_SCALE to undo it.
WH_SCALE = 64.0
# gate order inside a chunk: (i, f, o, j); column base offsets in the fused
# [*, 4H] kernel layout where reference order is i, j, f, o
GATE_BASE = [0, 2 * H, 3 * H, 1 * H]
SLOT = T  # h history slots per chunk (h_t at slot t; t=0 is special-cased)

# Matmul emission order: two phases (contraction chunks 0-1, then 2-3) so the
# gate chain of half the output chunks hides under the other half's weight
# streaming; within a phase, output-bank A (chunks 0,1) completes first.
# The last MOVE_LATE bank-B phase-0 pairs are deferred until after bank A's
# phase-1 block so bank A's stop-semaphore fires earlier each step.
MOVE_LATE = 3
MM_ORDER = []
_late = []
for _phase in (0, 1):
    for _bank in (0, 1):
        _blk = []
        for _ck in (2 * _phase, 2 * _phase + 1):
            for _c_rel in (0, 1):
                for _g in range(4):
                    _blk.append((_bank * 2 + _c_rel, _g, _ck))
        if _phase == 0 and _bank == 1:
            _late = _blk[-MOVE_LATE:]
            _blk = _blk[:-MOVE_LATE]
        MM_ORDER.extend(_blk)
        if _phase == 1 and _bank == 0:
            MM_ORDER.extend(_late)
_BANK_FIRST = {}
_BANK_LAST = {}
for _i, (_c, _g, _ck) in enumerate(MM_ORDER):
    _b = _c // 2
    _BANK_FIRST.setdefault(_b, _i)
    _BANK_LAST[_b] = _i

_CACHE = {}


def _build_program():
    from concourse import bacc
    import concourse.mybir as mybir
    from concourse.tile import TileContext

    f32 = mybir.dt.float32
    bf16 = mybir.dt.bfloat16
    fp8 = mybir.dt.float8e4
    AFT = mybir.ActivationFunctionType

    nc = bacc.Bacc(target_bir_lowering=False)

    e_d = nc.declare_dram_parameter("e", [128, (T // 4) * 2 * 128], bf16, False)
    i16_d = nc.declare_dram_parameter("i16", [128, 16], bf16, False)
    wh_d = nc.declare_dram_parameter("wh", [128, 64 * 128], fp8, False)
    w1_d = nc.declare_dram_parameter("w1", [128, 4 * DH], bf16, False)
    w2_d = nc.declare_dram_parameter("w2", [128, 2 * 4096], bf16, False)
    b1_d = nc.declare_dram_parameter("b1v", [128, 2], f32, False)
    b2_d = nc.declare_dram_parameter("b2v", [128, 32], f32, False)
    out_d = nc.declare_dram_parameter("out", [4096, 2 * T], f32, isOutput=True)

    with TileContext(nc) as tc:
        with tc.sbuf_pool(name="const", bufs=1) as cpool:
            e_sb = cpool.tile([128, (T // 4) * 2 * 128], bf16, name="e_sb")
            i16_sb = cpool.tile([128, 16], bf16, name="i16_sb")
            wh_sb = cpool.tile([128, 64 * 128], fp8, name="wh_sb")
            w1_sb = cpool.tile([128, 4 * DH], bf16, name="w1_sb")
            w2_sb = cpool.tile([128, 2 * 4096], bf16, name="w2_sb")
            b1_sb = cpool.tile([128, 2], f32, name="b1_sb")
            b2_sb = cpool.tile([128, 32], f32, name="b2_sb")
            # persistent state
            hst = cpool.tile([128, 4 * SLOT * 2], bf16, name="hst")
            cst = cpool.tile([128, 8], f32, name="cst")
            h1t = cpool.tile([128, 2048], bf16, name="h1t")

            # smallest-first so the biases' fence ops (which gate the chain
            # engines' clocks) and the first e quarter unblock ASAP; w2/e1-3
            # aren't read until hundreds of us in
            nc.sync.dma_start(out=b1_sb[:, :], in_=b1_d[:, :])
            nc.sync.dma_start(out=b2_sb[:, :], in_=b2_d[:, :])
            nc.sync.dma_start(out=i16_sb[:, :], in_=i16_d[:, :])
            nc.sync.dma_start(out=w1_sb[:, :], in_=w1_d[:, :])
            nc.sync.dma_start(out=wh_sb[:, :], in_=wh_d[:, :])
            ec = ((T // 4) * 2 * 128) // 4
            nc.sync.dma_start(out=e_sb[:, 0:ec], in_=e_d[:, 0:ec])
            nc.sync.dma_start(out=w2_sb[:, :], in_=w2_d[:, :])
            for i in range(1, 4):
                nc.sync.dma_start(
                    out=e_sb[:, i * ec:(i + 1) * ec], in_=e_d[:, i * ec:(i + 1) * ec]
                )

            # Engine-clock fences: each absorbs one input-DMA dependency into
            # an engine's observed clock so per-iteration ops carry at most a
            # single semaphore wait (walrus embedded-sync budget).
            fence = cpool.tile([128, 4], f32, name="fence")
            nc.vector.tensor_copy(fence[:, 0:1], e_sb[:, 0:1])
            # only fence the first e quarter: steps 0-127 read cols < ec, and
            # quarters 1-3 land ~400us before their first reader needs them
            nc.vector.tensor_copy(fence[:, 1:2], e_sb[:, ec - 1: ec])
            nc.vector.tensor_copy(fence[:, 2:3], b2_sb[:, 0:1])
            nc.scalar.add(fence[:, 3:4], b1_sb[:, 0:1], 0.0)
            nc.tensor.ldweights(wh_sb[:, 0:128])

            mult = mybir.AluOpType.mult
            addop = mybir.AluOpType.add
            subop = mybir.AluOpType.subtract

            NT = 2 * T  # MLP rows per core
            NF = min(128, NT)  # MLP row-block size
            NCH = NT // NF

            with tc.psum_pool(name="zp", bufs=2) as zpool, \
                    tc.sbuf_pool(name="gw", bufs=6) as gpool, \
                    tc.psum_pool(name="m1p", bufs=2) as m1pool, \
                    tc.psum_pool(name="m2p", bufs=2) as m2pool, \
                    tc.sbuf_pool(name="ob", bufs=8) as opool:

                def emit_m1(col0, w, ms=(0, 1)):
                    """hidden layer for MLP rows [col0, col0+w) of hst cols."""
                    for m in ms:
                        pm = 128 if m == 0 else DH - 128
                        ps = m1pool.tile([128, w], f32, tag="h1", name=f"h1ps{m}{col0}")
                        for c in range(4):
                            nc.tensor.matmul(
                                ps[0:pm, :],
                                w1_sb[:, c * DH + m * 128: c * DH + m * 128 + pm],
                                hst[:, c * SLOT * 2 + col0: c * SLOT * 2 + col0 + w],
                                start=(c == 0),
                                stop=(c == 3),
                            )
                        nc.scalar.activation(
                            h1t[0:pm, m * NT + col0: m * NT + col0 + w],
                            ps[0:pm, :],
                            AFT.Relu,
                            bias=b1_sb[0:pm, m:m + 1],
                        )

                def emit_o2(col0, w, mi):
                    """output rows [mi*128, (mi+1)*128) for MLP cols [col0, col0+w)."""
                    ps2 = m2pool.tile([128, w], f32, tag="o2", name=f"o2ps{mi}{col0}")
                    nc.tensor.matmul(
                        ps2[:, :],
                        w2_sb[:, mi * 128: mi * 128 + 128],
                        h1t[0:128, col0: col0 + w],
                        start=True, stop=False,
                    )
                    nc.tensor.matmul(
                        ps2[:, :],
                        w2_sb[0:72, 4096 + mi * 128: 4096 + mi * 128 + 128],
                        h1t[0:72, NT + col0: NT + col0 + w],
                        start=False, stop=True,
                    )
                    ob = opool.tile([128, w], f32, tag="ob", name=f"ob{mi}{col0}")
                    if mi % 2 == 0:
                        nc.vector.tensor_scalar_add(ob[:, :], ps2[:, :], b2_sb[:, mi:mi + 1])
                    else:
                        nc.scalar.add(ob[:, :], ps2[:, :], b2_sb[:, mi:mi + 1])
                    nc.sync.dma_start(
                        out=out_d[mi * 128:(mi + 1) * 128, col0: col0 + w],
                        in_=ob[:, :],
                    )

                # Spread MLP emission through the recurrence's idle gaps:
                # blocks 0-6 (128 cols each), then a 64-col half-block, leaving
                # only 5 output chunks + the last 64-col half-block for the
                # tail. sched: step -> list of emit thunks.
                sched = {}
                tail = []
                if T == 512:
                    for n in range(7):
                        base = 64 * n + 65
                        c0 = 128 * n
                        sched[base] = [lambda c0=c0: emit_m1(c0, 128, ms=(0,))]
                        sched[base + 1] = [lambda c0=c0: emit_m1(c0, 128, ms=(1,))]
                        for mi in range(32):
                            sched[base + 2 + mi] = [
                                lambda c0=c0, mi=mi: emit_o2(c0, 128, mi)
                            ]
                    sched[483] = [lambda: emit_m1(896, 64, ms=(0,))]
                    sched[484] = [lambda: emit_m1(896, 64, ms=(1,))]
                    for mi in range(32):
                        t_ = 485 + mi
                        th = lambda mi=mi: emit_o2(896, 64, mi)
                        if t_ < T:
                            sched[t_] = [th]
                        else:
                            tail.append(th)
                    tail.append(lambda: emit_m1(960, 64))
                    for mi in range(32):
                        tail.append(lambda mi=mi: emit_o2(960, 64, mi))
                else:
                    for n in range(NCH):
                        tail.append(lambda n=n: emit_m1(n * NF, NF))
                        for mi in range(32):
                            tail.append(lambda n=n, mi=mi: emit_o2(n * NF, NF, mi))

                import bass_rust as _br

                for t in range(T):
                    zps = [
                        zpool.tile([128, 16], f32, tag=f"z{b}", name=f"zp{b}")
                        for b in (0, 1)
                    ]
                    prev_stt2 = None
                    # The E (one-hot-embedding + bias) term enters through a
                    # K=16 identity matmul that also opens the bank's
                    # accumulation group (start=True writes all 16 columns);
                    # the 32 W_h matmuls then accumulate on top.
                    ebase = (t % 4) * 32
                    for b in (0, 1):
                        tcol = (t // 4) * 2 + b
                        nc.tensor.matmul(
                            zps[b][:, 0:16],
                            e_sb[ebase:ebase + 16, tcol * 128:(tcol + 1) * 128],
                            i16_sb[ebase:ebase + 16, 0:16],
                            start=True,
                            stop=(t == 0),
                            tile_position=(ebase, 0),
                        )
                    if t > 0:
                        for i, (c, g, ck) in enumerate(MM_ORDER):
                            b = c // 2
                            col = (c % 2) * 8 + g * 2
                            nc.tensor.matmul(
                                zps[b][:, col:col + 2],
                                wh_sb[:, i * 128:(i + 1) * 128],
                                hst[:, ck * SLOT * 2 + (t - 1) * 2: ck * SLOT * 2 + (t - 1) * 2 + 2],
                                start=False,
                                stop=(i == _BANK_LAST[b]),
                            )
                    g2_ops = []  # (t2, c_stt, h_tt) per chain
                    for g2 in (0, 1):
                        # gates for output chunks {2*g2, 2*g2+1}; layout per
                        # chunk: i(2) f(2) o(2) j(2); tanh(j) == 2*sig(2j)-1
                        # with the 2x folded into W/E host-side, so ONE
                        # sigmoid covers everything, reading PSUM directly.
                        sfj = gpool.tile([128, 16], f32, tag=f"s{g2}", name=f"sifoj{g2}")
                        nc.scalar.activation(
                            sfj[:, :], zps[g2][:, :], AFT.Sigmoid, scale=1.0 / WH_SCALE
                        )
                        s3 = sfj.rearrange("p (c x) -> p c x", c=2)
                        c3 = cst.rearrange("p (c x) -> p c x", c=4)[:, 2 * g2:2 * g2 + 2, :]
                        t1h = gpool.tile([128, 4], f32, tag=f"t1{g2}", name=f"t1h{g2}")
                        t13 = t1h.rearrange("p (c x) -> p c x", c=2)
                        # t1h = (sig(2j') - 0.5) * sig(i)   [= tanh(j)*sig(i)/2]
                        i1 = nc.vector.scalar_tensor_tensor(
                            t13[:, :, :], s3[:, :, 6:8], 0.5, s3[:, :, 0:2], subop, mult
                        )
                        if prev_stt2 is not None:
                            # keep group A's chain tail ahead of B's ops on DVE
                            _br.add_dep_helper(
                                i1.ins, prev_stt2.ins, sync=False,
                                reason="chain-A tail before chain-B start",
                            )
                        if t == 0:
                            # c_0 = 2 * t1h
                            prev_stt2 = nc.vector.tensor_scalar_mul(
                                c3[:, :, :], t13[:, :, :], 2.0
                            )
                        else:
                            t2 = gpool.tile([128, 4], f32, tag=f"t2{g2}", name=f"t2{g2}")
                            t23 = t2.rearrange("p (c x) -> p c x", c=2)
                            i2 = nc.vector.tensor_mul(t23[:, :, :], c3[:, :, :], s3[:, :, 2:4])
                            if prev_stt2 is not None:
                                _br.add_dep_helper(
                                    i2.ins, prev_stt2.ins, sync=False,
                                    reason="chain-A tail before chain-B t2",
                                )
                            # c = 2*t1h + c*sig(f)
                            prev_stt2 = nc.vector.scalar_tensor_tensor(
                                c3[:, :, :], t13[:, :, :], 2.0, t23[:, :, :], mult, addop
                            )
                        tcs = gpool.tile([128, 4], f32, tag=f"tc{g2}", name=f"tcs{g2}")
                        tc3 = tcs.rearrange("p (c x) -> p c x", c=2)
                        nc.scalar.activation(tc3[:, :, :], c3[:, :, :], AFT.Tanh)
                        h3 = hst.rearrange("p (c x) -> p c x", c=4)[
                            :, 2 * g2:2 * g2 + 2, t * 2:t * 2 + 2
                        ]
                        h_tt = nc.vector.tensor_mul(
                            h3[:, :, :], tc3[:, :, :], s3[:, :, 4:6]
                        )
                        g2_ops.append((i2 if t > 0 else None, prev_stt2, h_tt))
                    if t > 0:
                        # DVE order t2B -> hA -> cB: hA (critical: gates the
                        # next step's matmuls) must not queue behind cB
                        hA = g2_ops[0][2]
                        _br.add_dep_helper(
                            hA.ins, g2_ops[1][0].ins, sync=False,
                            reason="hA after t2B",
                        )
                        _br.add_dep_helper(
                            g2_ops[1][1].ins, hA.ins, sync=False,
                            reason="cB after hA",
                        )
                    for thunk in sched.get(t, ()):
                        thunk()
                for thunk in tail:
                    thunk()
    nc.finalize()
    return nc


def _prep_host(tokens, lstm_kernel, lstm_bias, W1, b1, W2, b2):
    """Build per-core input arrays in the packed layouts the program expects."""
    bf = ml_dtypes.bfloat16
    tokens = np.asarray(tokens)
    lstm_kernel = np.asarray(lstm_kernel, dtype=np.float32)
    lstm_bias = np.asarray(lstm_bias, dtype=np.float32)
    W1 = np.asarray(W1, dtype=np.float32)
    b1 = np.asarray(b1, dtype=np.float32)
    W2 = np.asarray(W2, dtype=np.float32)
    b2 = np.asarray(b2, dtype=np.float32)

    Wx = lstm_kernel[:V]
    Wh = lstm_kernel[V:]
    bias = lstm_bias.copy()
    bias[2 * H:3 * H] += 1.0  # forget-gate bias (i, j, f, o layout)

    # tanh(j) is computed as 2*sig(2j)-1: double the j-gate columns (exact in
    # bf16) so one sigmoid covers all four gates.
    jsl = slice(H, 2 * H)  # j block in the (i, j, f, o) fused layout

    # permuted z-dim order: dim' = (c*4+g)*128 + p  ->  GATE_BASE[g] + c*128 + p
    perm = np.empty(4 * H, dtype=np.int64)
    for c in range(4):
        for g in range(4):
            mt = c * 4 + g
            perm[mt * 128:(mt + 1) * 128] = GATE_BASE[g] + c * 128 + np.arange(128)

    # E with bias folded, gathered per core:
    # e[p, t*32 + c*8 + g*2 + b] = (Wx[tok[b,t]] + bias)[GATE_BASE[g] + c*128 + p]
    Wx_adj = (Wx + bias[None, :]) * WH_SCALE
    Wx_adj[:, jsl] *= 2.0
    Wx_adj = Wx_adj.astype(bf)                        # [V, 4H]
    Wx_re = np.ascontiguousarray(Wx_adj[:, perm])     # [V, (c,g,p) = ((c*4+g)*128+p)]

    # wh tile i (in MM_ORDER) = Wh[ck*128:(ck+1)*128, GATE_BASE[g]+c*128 ...]
    fp8np = ml_dtypes.float8_e4m3
    Whs = Wh * WH_SCALE
    Whs[:, jsl] *= 2.0
    Whb = Whs.astype(fp8np)
    wh = np.empty((128, 64 * 128), dtype=fp8np)
    for i, (c, g, ck) in enumerate(MM_ORDER):
        wh[:, i * 128:(i + 1) * 128] = Whb[
            ck * 128:(ck + 1) * 128, GATE_BASE[g] + c * 128: GATE_BASE[g] + (c + 1) * 128
        ]

    # w1[p, c*DH + d] = (W1 * BN_S)[c*128 + p, d]
    W1s = (W1 * BN_S).astype(bf)
    w1 = np.empty((128, 4 * DH), dtype=bf)
    for c in range(4):
        w1[:, c * DH:(c + 1) * DH] = W1s[c * 128:(c + 1) * 128, :]

    # W2 with BN scale folded; b2 applied separately in fp32
    W2s = (W2 * BN_S).astype(bf)
    w2 = np.zeros((128, 2 * 4096), dtype=bf)
    w2[:, :4096] = W2s[0:128, :]
    w2[0:72, 4096:] = W2s[128:200, :]

    b1v = np.zeros((128, 2), dtype=np.float32)
    b1v[:, 0] = b1[0:128]
    b1v[0:72, 1] = b1[128:200]
    b2v = np.ascontiguousarray((b2 * BN_S).reshape(32, 128).T.astype(np.float32))

    # identity blocks for the E-injection matmul, at partition bases 0/32/64/96
    i16 = np.zeros((128, 16), dtype=bf)
    for bb in range(4):
        for kk in range(16):
            i16[bb * 32 + kk, kk] = 1.0

    in_maps = []
    for k in range(N_CORES):
        tok_core = tokens[2 * k:2 * k + 2, :T].astype(np.int64)  # [2, T]
        g_ = Wx_re[tok_core.reshape(-1)].reshape(2, T, 4, 4, 128)  # [b, t, c, g, p]
        A = np.transpose(g_, (1, 2, 3, 0, 4)).reshape(T, 2, 16, 128)  # [t, bank, row(c_rel,g,b), p]
        # e[(t%4)*32 + row, ((t//4)*2 + bank)*128 + p] = A[t, bank, row, p]
        e = np.zeros((128, (T // 4) * 2 * 128), dtype=bf)
        for r4 in range(4):
            sel = A[r4::4]  # [T//4, bank, row, p]
            e[r4 * 32: r4 * 32 + 16, :] = np.transpose(sel, (2, 0, 1, 3)).reshape(
                16, (T // 4) * 2 * 128
            )
        in_maps.append({
            "e": e,
            "i16": i16,
            "wh": wh,
            "w1": w1,
            "w2": w2,
            "b1v": b1v,
            "b2v": b2v,
        })
    return in_maps


def kernel(tokens, lstm_kernel, lstm_bias, W1, b1, W2, b2):
    from concourse.bass_utils import run_bass_kernel_spmd

    if "nc" not in _CACHE:
        _CACHE["nc"] = _build_program()
    nc = _CACHE["nc"]

    in_maps = _prep_host(tokens, lstm_kernel, lstm_bias, W1, b1, W2, b2)
    res = run_bass_kernel_spmd(nc, in_maps, list(range(N_CORES)))
    results = res.results

    out = np.empty((B * T, V), dtype=np.float32)
    for k in range(N_CORES):
        o = np.asarray(results[k]["out"], dtype=np.float32)  # [4096, 2T] (v, t*2+b)
        o = o.reshape(V, T, 2)
        out[(2 * k) * T:(2 * k + 1) * T, :] = o[:, :, 0].T
        out[(2 * k + 1) * T:(2 * k + 2) * T, :] = o[:, :, 1].T
    return out



# revision 29
# speedup vs baseline: 1.2322x; 1.2322x over previous
"""Trainium2 Bass kernel for nn_Network_80049600463282.

LSTM language model: B=16, T=512, V=4096 (one-hot input), H=512 LSTM,
then MLP 512 -> 200 (relu) -> 4096, with fixed batch-norm scaling.

Strategy (8 NeuronCores, SPMD, zero collectives):
  - Data-parallel over batch: 2 examples per core.
  - One-hot @ W_x == embedding row gather -> precomputed host-side into the
    exact transposed SBUF layout the recurrence consumes (lstm bias + forget
    bias folded in).
  - Recurrence in transposed layout (gate dims on partitions): per step,
    64 bf16 matmuls [128x128 stationary W_h tile] x [128x2 moving h^T],
    PSUM-accumulated per 128-dim output chunk; sigmoid/tanh on ScalarE,
    cell update on VectorE, chunked x4 so gate latency hides under the
    next chunk's weight streaming.
  - hs^T accumulates in SBUF in matmul-ready layout; MLP runs at the end
    with BN scales folded into W1/W2 and b2 folded in via K=201 augmentation.
"""

import os
import numpy as np
import ml_dtypes

V = 4096
B = 16
T = int(os.environ.get("LSTM_KERNEL_T", "512"))
H = 512
DH = 200
N_CORES = 8
BL = 2  # examples per core
BN_S = 1.0 / np.sqrt(1.0 + 0.001)
# W_h is stored fp8e4m3 (FWL streams fp8 weights 4 cols/cycle vs bf16's 2).
# Weights and the E term are pre-scaled by WH_SCALE so fp8 stays in its
# normal range; the sigmoid applies 1/WH_SCALE to undo it.
WH_SCALE = 64.0
# gate order inside a chunk: (i, f, o, j); column base offsets in the fused
# [*, 4H] kernel layout where reference order is i, j, f, o
GATE_BASE = [0, 2 * H, 3 * H, 1 * H]
SLOT = T  # h history slots per chunk (h_t at slot t; t=0 is special-cased)

# Matmul emission order: two phases (contraction chunks 0-1, then 2-3) so the
# gate chain of half the output chunks hides under the other half's weight
# streaming; within a phase, output-bank A (chunks 0,1) completes first.
# A2-early layout: bank A's phase-1 block runs right after h chunks 2,3
# arrive, so bank A completes (and its gate chain starts) ~16 pairs earlier;
# most of bank B's phase-0 pairs fill the stream after it. PIN_AFTER marks
# (index, index-it-must-follow) pairs handed to add_dep to keep the scheduler
# from floating bank-B work back before bank A's stop.
def _mk_order():
    def blk(bank, cks):
        return [
            (bank * 2 + c_rel, g, ck)
            for ck in cks
            for c_rel in (0, 1)
            for g in range(4)
        ]
    a01 = blk(0, (0, 1))
    b01 = blk(1, (0, 1))
    a23 = blk(0, (2, 3))
    b23 = blk(1, (2, 3))
    order = a01 + b01[:7] + a23 + b01[7:] + b23
    pins = []
    # keep A's phase-1 after the early B pairs, and the late B pairs after
    # A's stop, so the schedule can't collapse back to the default interleave
    i_a23_first = len(a01) + 7
    i_a23_last = i_a23_first + len(a23) - 1
    pins.append((i_a23_first, i_a23_first - 1))
    pins.append((i_a23_last + 1, i_a23_last))
    return order, dict(pins)

MM_ORDER, PIN_AFTER = _mk_order()
_BANK_FIRST = {}
_BANK_LAST = {}
for _i, (_c, _g, _ck) in enumerate(MM_ORDER):
    _b = _c // 2
    _BANK_FIRST.setdefault(_b, _i)
    _BANK_LAST[_b] = _i

_CACHE = {}


def _build_program():
    from concourse import bacc
    import concourse.mybir as mybir
    from concourse.tile import TileContext

    f32 = mybir.dt.float32
    bf16 = mybir.dt.bfloat16
    fp8 = mybir.dt.float8e4
    AFT = mybir.ActivationFunctionType

    nc = bacc.Bacc(target_bir_lowering=False)

    e_d = nc.declare_dram_parameter("e", [128, (T // 4) * 2 * 128], bf16, False)
    i16_d = nc.declare_dram_parameter("i16", [128, 16], bf16, False)
    wh_d = nc.declare_dram_parameter("wh", [128, 64 * 128], fp8, False)
    w1_d = nc.declare_dram_parameter("w1", [128, 4 * DH], bf16, False)
    w2_d = nc.declare_dram_parameter("w2", [128, 2 * 4096], bf16, False)
    b1_d = nc.declare_dram_parameter("b1v", [128, 2], f32, False)
    b2_d = nc.declare_dram_parameter("b2v", [128, 32], f32, False)
    out_d = nc.declare_dram_parameter("out", [4096, 2 * T], f32, isOutput=True)

    with TileContext(nc) as tc:
        with tc.sbuf_pool(name="const", bufs=1) as cpool:
            e_sb = cpool.tile([128, (T // 4) * 2 * 128], bf16, name="e_sb")
            i16_sb = cpool.tile([128, 16], bf16, name="i16_sb")
            wh_sb = cpool.tile([128, 64 * 128], fp8, name="wh_sb")
            w1_sb = cpool.tile([128, 4 * DH], bf16, name="w1_sb")
            w2_sb = cpool.tile([128, 2 * 4096], bf16, name="w2_sb")
            b1_sb = cpool.tile([128, 2], f32, name="b1_sb")
            b2_sb = cpool.tile([128, 32], f32, name="b2_sb")
            # persistent state
            hst = cpool.tile([128, 4 * SLOT * 2], bf16, name="hst")
            cst = cpool.tile([128, 8], f32, name="cst")
            h1t = cpool.tile([128, 2048], bf16, name="h1t")

            # smallest-first so the biases' fence ops (which gate the chain
            # engines' clocks) and the first e quarter unblock ASAP; w2/e1-3
            # aren't read until hundreds of us in
            nc.sync.dma_start(out=b1_sb[:, :], in_=b1_d[:, :])
            nc.sync.dma_start(out=b2_sb[:, :], in_=b2_d[:, :])
            nc.sync.dma_start(out=i16_sb[:, :], in_=i16_d[:, :])
            nc.sync.dma_start(out=w1_sb[:, :], in_=w1_d[:, :])
            nc.sync.dma_start(out=wh_sb[:, :], in_=wh_d[:, :])
            ec = ((T // 4) * 2 * 128) // 4
            nc.sync.dma_start(out=e_sb[:, 0:ec], in_=e_d[:, 0:ec])
            nc.sync.dma_start(out=w2_sb[:, :], in_=w2_d[:, :])
            for i in range(1, 4):
                nc.sync.dma_start(
                    out=e_sb[:, i * ec:(i + 1) * ec], in_=e_d[:, i * ec:(i + 1) * ec]
                )

            # Engine-clock fences: each absorbs one input-DMA dependency into
            # an engine's observed clock so per-iteration ops carry at most a
            # single semaphore wait (walrus embedded-sync budget).
            fence = cpool.tile([128, 4], f32, name="fence")
            nc.vector.tensor_copy(fence[:, 0:1], e_sb[:, 0:1])
            # only fence the first e quarter: steps 0-127 read cols < ec, and
            # quarters 1-3 land ~400us before their first reader needs them
            nc.vector.tensor_copy(fence[:, 1:2], e_sb[:, ec - 1: ec])
            nc.vector.tensor_copy(fence[:, 2:3], b2_sb[:, 0:1])
            nc.scalar.add(fence[:, 3:4], b1_sb[:, 0:1], 0.0)
            nc.tensor.ldweights(wh_sb[:, 0:128])

            mult = mybir.AluOpType.mult
            addop = mybir.AluOpType.add
            subop = mybir.AluOpType.subtract

            NT = 2 * T  # MLP rows per core
            NF = min(128, NT)  # MLP row-block size
            NCH = NT // NF

            with tc.psum_pool(name="zp", bufs=2) as zpool, \
                    tc.sbuf_pool(name="gw", bufs=6) as gpool, \
                    tc.psum_pool(name="m1p", bufs=2) as m1pool, \
                    tc.psum_pool(name="m2p", bufs=2) as m2pool, \
                    tc.sbuf_pool(name="ob", bufs=8) as opool:

                def emit_m1(col0, w, ms=(0, 1)):
                    """hidden layer for MLP rows [col0, col0+w) of hst cols."""
                    for m in ms:
                        pm = 128 if m == 0 else DH - 128
                        ps = m1pool.tile([128, w], f32, tag="h1", name=f"h1ps{m}{col0}")
                        for c in range(4):
                            nc.tensor.matmul(
                                ps[0:pm, :],
                                w1_sb[:, c * DH + m * 128: c * DH + m * 128 + pm],
                                hst[:, c * SLOT * 2 + col0: c * SLOT * 2 + col0 + w],
                                start=(c == 0),
                                stop=(c == 3),
                            )
                        nc.scalar.activation(
                            h1t[0:pm, m * NT + col0: m * NT + col0 + w],
                            ps[0:pm, :],
                            AFT.Relu,
                            bias=b1_sb[0:pm, m:m + 1],
                        )

                def emit_o2(col0, w, mi):
                    """output rows [mi*128, (mi+1)*128) for MLP cols [col0, col0+w)."""
                    ps2 = m2pool.tile([128, w], f32, tag="o2", name=f"o2ps{mi}{col0}")
                    nc.tensor.matmul(
                        ps2[:, :],
                        w2_sb[:, mi * 128: mi * 128 + 128],
                        h1t[0:128, col0: col0 + w],
                        start=True, stop=False,
                    )
                    nc.tensor.matmul(
                        ps2[:, :],
                        w2_sb[0:72, 4096 + mi * 128: 4096 + mi * 128 + 128],
                        h1t[0:72, NT + col0: NT + col0 + w],
                        start=False, stop=True,
                    )
                    ob = opool.tile([128, w], f32, tag="ob", name=f"ob{mi}{col0}")
                    if mi % 2 == 0:
                        nc.vector.tensor_scalar_add(ob[:, :], ps2[:, :], b2_sb[:, mi:mi + 1])
                    else:
                        nc.scalar.add(ob[:, :], ps2[:, :], b2_sb[:, mi:mi + 1])
                    nc.sync.dma_start(
                        out=out_d[mi * 128:(mi + 1) * 128, col0: col0 + w],
                        in_=ob[:, :],
                    )

                # Spread MLP emission through the recurrence's idle gaps:
                # blocks 0-6 (128 cols each), then a 64-col half-block, leaving
                # only 5 output chunks + the last 64-col half-block for the
                # tail. sched: step -> list of emit thunks.
                sched = {}
                tail = []
                if T == 512:
                    for n in range(7):
                        base = 64 * n + 65
                        c0 = 128 * n
                        sched[base] = [lambda c0=c0: emit_m1(c0, 128, ms=(0,))]
                        sched[base + 1] = [lambda c0=c0: emit_m1(c0, 128, ms=(1,))]
                        for mi in range(32):
                            sched[base + 2 + mi] = [
                                lambda c0=c0, mi=mi: emit_o2(c0, 128, mi)
                            ]
                    sched[483] = [lambda: emit_m1(896, 64, ms=(0,))]
                    sched[484] = [lambda: emit_m1(896, 64, ms=(1,))]
                    for mi in range(32):
                        t_ = 485 + mi
                        th = lambda mi=mi: emit_o2(896, 64, mi)
                        if t_ < T:
                            sched[t_] = [th]
                        else:
                            tail.append(th)
                    tail.append(lambda: emit_m1(960, 64))
                    for mi in range(32):
                        tail.append(lambda mi=mi: emit_o2(960, 64, mi))
                else:
                    for n in range(NCH):
                        tail.append(lambda n=n: emit_m1(n * NF, NF))
                        for mi in range(32):
                            tail.append(lambda n=n, mi=mi: emit_o2(n * NF, NF, mi))

                import bass_rust as _br

                for t in range(T):
                    zps = [
                        zpool.tile([128, 16], f32, tag=f"z{b}", name=f"zp{b}")
                        for b in (0, 1)
                    ]
                    prev_stt2 = None
                    # The E (one-hot-embedding + bias) term enters through a
                    # K=16 identity matmul that also opens the bank's
                    # accumulation group (start=True writes all 16 columns);
                    # the 32 W_h matmuls then accumulate on top.
                    ebase = (t % 4) * 32
                    for b in (0, 1):
                        tcol = (t // 4) * 2 + b
                        nc.tensor.matmul(
                            zps[b][:, 0:16],
                            e_sb[ebase:ebase + 16, tcol * 128:(tcol + 1) * 128],
                            i16_sb[ebase:ebase + 16, 0:16],
                            start=True,
                            stop=(t == 0),
                            tile_position=(ebase, 0),
                        )
                    if t > 0:
                        mms = []
                        for i, (c, g, ck) in enumerate(MM_ORDER):
                            b = c // 2
                            col = (c % 2) * 8 + g * 2
                            mm = nc.tensor.matmul(
                                zps[b][:, col:col + 2],
                                wh_sb[:, i * 128:(i + 1) * 128],
                                hst[:, ck * SLOT * 2 + (t - 1) * 2: ck * SLOT * 2 + (t - 1) * 2 + 2],
                                start=False,
                                stop=(i == _BANK_LAST[b]),
                            )
                            mms.append(mm)
                            if i in PIN_AFTER:
                                _br.add_dep_helper(
                                    mm.ins, mms[PIN_AFTER[i]].ins, sync=False,
                                    reason="pin MM order",
                                )
                    g2_ops = []  # (t2, c_stt, h_tt) per chain
                    for g2 in (0, 1):
                        # gates for output chunks {2*g2, 2*g2+1}; layout per
                        # chunk: i(2) f(2) o(2) j(2); tanh(j) == 2*sig(2j)-1
                        # with the 2x folded into W/E host-side, so ONE
                        # sigmoid covers everything, reading PSUM directly.
                        sfj = gpool.tile([128, 16], f32, tag=f"s{g2}", name=f"sifoj{g2}")
                        nc.scalar.activation(
                            sfj[:, :], zps[g2][:, :], AFT.Sigmoid, scale=1.0 / WH_SCALE
                        )
                        s3 = sfj.rearrange("p (c x) -> p c x", c=2)
                        c3 = cst.rearrange("p (c x) -> p c x", c=4)[:, 2 * g2:2 * g2 + 2, :]
                        t1h = gpool.tile([128, 4], f32, tag=f"t1{g2}", name=f"t1h{g2}")
                        t13 = t1h.rearrange("p (c x) -> p c x", c=2)
                        # t1h = (sig(2j') - 0.5) * sig(i)   [= tanh(j)*sig(i)/2]
                        i1 = nc.vector.scalar_tensor_tensor(
                            t13[:, :, :], s3[:, :, 6:8], 0.5, s3[:, :, 0:2], subop, mult
                        )
                        if prev_stt2 is not None:
                            # keep group A's chain tail ahead of B's ops on DVE
                            _br.add_dep_helper(
                                i1.ins, prev_stt2.ins, sync=False,
                                reason="chain-A tail before chain-B start",
                            )
                        if t == 0:
                            # c_0 = 2 * t1h
                            prev_stt2 = nc.vector.tensor_scalar_mul(
                                c3[:, :, :], t13[:, :, :], 2.0
                            )
                        else:
                            t2 = gpool.tile([128, 4], f32, tag=f"t2{g2}", name=f"t2{g2}")
                            t23 = t2.rearrange("p (c x) -> p c x", c=2)
                            i2 = nc.vector.tensor_mul(t23[:, :, :], c3[:, :, :], s3[:, :, 2:4])
                            if prev_stt2 is not None:
                                _br.add_dep_helper(
                                    i2.ins, prev_stt2.ins, sync=False,
                                    reason="chain-A tail before chain-B t2",
                                )
                            # c = 2*t1h + c*sig(f)
                            prev_stt2 = nc.vector.scalar_tensor_tensor(
                                c3[:, :, :], t13[:, :, :], 2.0, t23[:, :, :], mult, addop
                            )
                        tcs = gpool.tile([128, 4], f32, tag=f"tc{g2}", name=f"tcs{g2}")
                        tc3 = tcs.rearrange("p (c x) -> p c x", c=2)
                        nc.scalar.activation(tc3[:, :, :], c3[:, :, :], AFT.Tanh)
                        h3 = hst.rearrange("p (c x) -> p c x", c=4)[
                            :, 2 * g2:2 * g2 + 2, t * 2:t * 2 + 2
                        ]
                        h_tt = nc.vector.tensor_mul(
                            h3[:, :, :], tc3[:, :, :], s3[:, :, 4:6]
                        )
                        g2_ops.append((i2 if t > 0 else None, prev_stt2, h_tt))
                    if t > 0:
                        # hA (critical: gates the next step's matmuls) must
                        # not queue behind chain B's cB on the DVE
                        hA = g2_ops[0][2]
                        _br.add_dep_helper(
                            g2_ops[1][1].ins, hA.ins, sync=False,
                            reason="cB after hA",
                        )
                    for thunk in sched.get(t, ()):
                        thunk()
                for thunk in tail:
                    thunk()
    nc.finalize()
    return nc


def _prep_host(tokens, lstm_kernel, lstm_bias, W1, b1, W2, b2):
    """Build per-core input arrays in the packed layouts the program expects."""
    bf = ml_dtypes.bfloat16
    tokens = np.asarray(tokens)
    lstm_kernel = np.asarray(lstm_kernel, dtype=np.float32)
    lstm_bias = np.asarray(lstm_bias, dtype=np.float32)
    W1 = np.asarray(W1, dtype=np.float32)
    b1 = np.asarray(b1, dtype=np.float32)
    W2 = np.asarray(W2, dtype=np.float32)
    b2 = np.asarray(b2, dtype=np.float32)

    Wx = lstm_kernel[:V]
    Wh = lstm_kernel[V:]
    bias = lstm_bias.copy()
    bias[2 * H:3 * H] += 1.0  # forget-gate bias (i, j, f, o layout)

    # tanh(j) is computed as 2*sig(2j)-1: double the j-gate columns (exact in
    # bf16) so one sigmoid covers all four gates.
    jsl = slice(H, 2 * H)  # j block in the (i, j, f, o) fused layout

    # permuted z-dim order: dim' = (c*4+g)*128 + p  ->  GATE_BASE[g] + c*128 + p
    perm = np.empty(4 * H, dtype=np.int64)
    for c in range(4):
        for g in range(4):
            mt = c * 4 + g
            perm[mt * 128:(mt + 1) * 128] = GATE_BASE[g] + c * 128 + np.arange(128)

    # E with bias folded, gathered per core:
    # e[p, t*32 + c*8 + g*2 + b] = (Wx[tok[b,t]] + bias)[GATE_BASE[g] + c*128 + p]
    Wx_adj = (Wx + bias[None, :]) * WH_SCALE
    Wx_adj[:, jsl] *= 2.0
    Wx_adj = Wx_adj.astype(bf)                        # [V, 4H]
    Wx_re = np.ascontiguousarray(Wx_adj[:, perm])     # [V, (c,g,p) = ((c*4+g)*128+p)]

    # wh tile i (in MM_ORDER) = Wh[ck*128:(ck+1)*128, GATE_BASE[g]+c*128 ...]
    fp8np = ml_dtypes.float8_e4m3
    Whs = Wh * WH_SCALE
    Whs[:, jsl] *= 2.0
    Whb = Whs.astype(fp8np)
    wh = np.empty((128, 64 * 128), dtype=fp8np)
    for i, (c, g, ck) in enumerate(MM_ORDER):
        wh[:, i * 128:(i + 1) * 128] = Whb[
            ck * 128:(ck + 1) * 128, GATE_BASE[g] + c * 128: GATE_BASE[g] + (c + 1) * 128
        ]

    # w1[p, c*DH + d] = (W1 * BN_S)[c*128 + p, d]
    W1s = (W1 * BN_S).astype(bf)
    w1 = np.empty((128, 4 * DH), dtype=bf)
    for c in range(4):
        w1[:, c * DH:(c + 1) * DH] = W1s[c * 128:(c + 1) * 128, :]

    # W2 with BN scale folded; b2 applied separately in fp32
    W2s = (W2 * BN_S).astype(bf)
    w2 = np.zeros((128, 2 * 4096), dtype=bf)
    w2[:, :4096] = W2s[0:128, :]
    w2[0:72, 4096:] = W2s[128:200, :]

    b1v = np.zeros((128, 2), dtype=np.float32)
    b1v[:, 0] = b1[0:128]
    b1v[0:72, 1] = b1[128:200]
    b2v = np.ascontiguousarray((b2 * BN_S).reshape(32, 128).T.astype(np.float32))

    # identity blocks for the E-injection matmul, at partition bases 0/32/64/96
    i16 = np.zeros((128, 16), dtype=bf)
    for bb in range(4):
        for kk in range(16):
            i16[bb * 32 + kk, kk] = 1.0

    in_maps = []
    for k in range(N_CORES):
        tok_core = tokens[2 * k:2 * k + 2, :T].astype(np.int64)  # [2, T]
        g_ = Wx_re[tok_core.reshape(-1)].reshape(2, T, 4, 4, 128)  # [b, t, c, g, p]
        A = np.transpose(g_, (1, 2, 3, 0, 4)).reshape(T, 2, 16, 128)  # [t, bank, row(c_rel,g,b), p]
        # e[(t%4)*32 + row, ((t//4)*2 + bank)*128 + p] = A[t, bank, row, p]
        e = np.zeros((128, (T // 4) * 2 * 128), dtype=bf)
        for r4 in range(4):
            sel = A[r4::4]  # [T//4, bank, row, p]
            e[r4 * 32: r4 * 32 + 16, :] = np.transpose(sel, (2, 0, 1, 3)).reshape(
                16, (T // 4) * 2 * 128
            )
        in_maps.append({
            "e": e,
            "i16": i16,
            "wh": wh,
            "w1": w1,
            "w2": w2,
            "b1v": b1v,
            "b2v": b2v,
        })
    return in_maps


def kernel(tokens, lstm_kernel, lstm_bias, W1, b1, W2, b2):
    from concourse.bass_utils import run_bass_kernel_spmd

    if "nc" not in _CACHE:
        _CACHE["nc"] = _build_program()
    nc = _CACHE["nc"]

    in_maps = _prep_host(tokens, lstm_kernel, lstm_bias, W1, b1, W2, b2)
    res = run_bass_kernel_spmd(nc, in_maps, list(range(N_CORES)))
    results = res.results

    out = np.empty((B * T, V), dtype=np.float32)
    for k in range(N_CORES):
        o = np.asarray(results[k]["out"], dtype=np.float32)  # [4096, 2T] (v, t*2+b)
        o = o.reshape(V, T, 2)
        out[(2 * k) * T:(2 * k + 1) * T, :] = o[:, :, 0].T
        out[(2 * k + 1) * T:(2 * k + 2) * T, :] = o[:, :, 1].T
    return out

